# revision 7
# baseline (speedup 1.0000x reference)
"""Multi-head attention (B=4, L=1024, E=1024, H=16) on 8 Trainium2 cores.

Sharding (SPMD, one program, per-core data): core c handles batch b = c//2 and
the 8-head half h0 = 8*(c%2).  Per core:
  - project q/k/v for its batch restricted to its 512-wide embed slice,
    producing TRANSPOSED activations qsT/ksT [e_local, i] and natural vs [j, e_local]
  - normal-layout logits (K=64 matmul + K=1 mask-row matmul) -> exp (accum_out
    row sums) -> normalize -> attn output slice [8, 1024, 1024]
  - transposed logits -> exp (mask as per-partition bias) -> attn@V accumulation
    (unnormalized, col-packed head pairs) -> scale by broadcast 1/s -> outTn
  - final projection partial out = outTn.T @ WoT (+bo on even cores only);
    host sums the two partials per batch.

Matmuls run in float32r (full-rate fp32, ~1.5e-4 relative error).  f32r
operands must be produced by rounding-capable instructions: DMA from
f32r-declared DRAM, ACT activations with f32r output, or gpsimd cast-DMAs.

Softmax is computed without max-subtraction (logits for these inputs are small;
masked entries get -9e15 added via the mask row / bias, exp underflows to
exactly 0).  This matches jax.nn.softmax to fp32 rounding.
"""

import numpy as np

import concourse.bass as bass
import concourse.mybir as mybir
import concourse.tile as tile
from concourse import bacc
from concourse.bass_utils import run_bass_kernel_spmd
from concourse.masks import make_identity

B, L, E, H, D = 4, 1024, 1024, 16, 64
HPC = 8               # heads per core
ES = HPC * D          # 512-wide embed slice per core
SCALE = float(D) ** -0.5
NEG = np.float32(-9e15)
F32 = mybir.dt.float32
F32R = mybir.dt.float32r
EXP = mybir.ActivationFunctionType.Exp
IDENT = mybir.ActivationFunctionType.Identity
ADD = mybir.AluOpType.add
MULT = mybir.AluOpType.mult


def _bcast_ap(ap, n_part):
    """Prepend a 0-step partition-broadcast dim to a DRAM AP."""
    return bass.AP(tensor=ap.tensor, offset=ap.offset,
                   ap=[[0, n_part]] + [list(d) for d in ap.ap])


def _build():
    nc = bacc.Bacc()

    qT = nc.dram_tensor("qT", [E, L], F32R, kind="ExternalInput")
    kT = nc.dram_tensor("kT", [E, L], F32R, kind="ExternalInput")
    vT = nc.dram_tensor("vT", [E, L], F32R, kind="ExternalInput")
    wqT = nc.dram_tensor("wqT", [E, ES], F32R, kind="ExternalInput")
    wkT = nc.dram_tensor("wkT", [E, ES], F32R, kind="ExternalInput")
    wvT = nc.dram_tensor("wvT", [E, ES], F32R, kind="ExternalInput")
    woT = nc.dram_tensor("woT", [ES, E], F32R, kind="ExternalInput")
    bq = nc.dram_tensor("bq", [ES], F32, kind="ExternalInput")
    bk = nc.dram_tensor("bk", [ES], F32, kind="ExternalInput")
    bv = nc.dram_tensor("bv", [ES], F32R, kind="ExternalInput")
    bo = nc.dram_tensor("bo", [E], F32R, kind="ExternalInput")
    maddp = nc.dram_tensor("maddp", [L], F32, kind="ExternalInput")
    maddp_r = nc.dram_tensor("maddp_r", [L], F32R, kind="ExternalInput")
    ones_d = nc.dram_tensor("ones_d", [128], F32R, kind="ExternalInput")

    attn_o = nc.dram_tensor("attn_o", [HPC, L, L], F32, kind="ExternalOutput")
    final_o = nc.dram_tensor("final_o", [L, E], F32, kind="ExternalOutput")

    NKC = E // 128        # 8 contraction chunks for projections
    NIC = L // 128        # 8 row chunks
    NT = L // 512         # 2 free tiles of 512
    NP = ES // 128        # 4 head-pair chunks

    with tile.TileContext(nc) as tc:
        with (
            tc.tile_pool(name="persist", bufs=1) as persist,
            tc.tile_pool(name="wstream", bufs=10) as wstream,
            tc.tile_pool(name="xstream", bufs=8) as xstream,
            tc.tile_pool(name="expP", bufs=4) as expP_pool,
            tc.tile_pool(name="expT", bufs=4) as expT_pool,
            tc.tile_pool(name="attn_st", bufs=3) as attn_st,
            tc.tile_pool(name="rbc", bufs=3) as rbc_pool,
            tc.tile_pool(name="fin", bufs=3) as fin_pool,
            tc.tile_pool(name="ps", bufs=4, space="PSUM") as ps_pool,
            tc.tile_pool(name="acc", bufs=4, space="PSUM") as acc_pool,
            tc.tile_pool(name="rdram", bufs=1, space="DRAM") as rdram,
        ):
            # ---- constants / small tiles ----
            ident = persist.tile([128, 128], F32, tag="ident")
            make_identity(nc, ident[:])
            ones1 = persist.tile([1, 128], F32R, tag="ones1")
            nc.sync.dma_start(ones1[:], ones_d[None, :])
            madd_row = persist.tile([1, L], F32R, tag="madd_row")
            nc.sync.dma_start(madd_row[:], maddp_r[None, :])
            madd_col = persist.tile([128, NIC], F32, tag="madd_col")
            nc.sync.dma_start(
                madd_col[:], maddp.rearrange("(jc p) -> p jc", p=128))
            bq_sb = persist.tile([128, NP], F32, tag="bq")
            nc.sync.dma_start(bq_sb[:], bq.rearrange("(mc p) -> p mc", p=128))
            bqs_sb = persist.tile([128, NP], F32, tag="bqs")
            nc.scalar.mul(bqs_sb[:], bq_sb[:], SCALE)
            bk_sb = persist.tile([128, NP], F32, tag="bk")
            nc.sync.dma_start(bk_sb[:], bk.rearrange("(mc p) -> p mc", p=128))
            bv_row = persist.tile([1, ES], F32R, tag="bv")
            nc.sync.dma_start(bv_row[:], bv[None, :])
            bo_row = persist.tile([1, E], F32R, tag="bo")
            nc.sync.dma_start(bo_row[:], bo[None, :])

            qsT = [persist.tile([128, L], F32R, tag=f"qsT{t}", name=f"qsT{t}")
                   for t in range(NP)]
            ksT = [persist.tile([128, L], F32R, tag=f"ksT{t}", name=f"ksT{t}")
                   for t in range(NP)]
            vs = [persist.tile([128, ES], F32R, tag=f"vs{j}", name=f"vs{j}")
                  for j in range(NIC)]
            outTn = [persist.tile([128, L], F32R, tag=f"outTn{t}", name=f"outTn{t}")
                     for t in range(NP)]
            woT_sb = [persist.tile([128, E], F32R, tag=f"woT{t}", name=f"woT{t}")
                      for t in range(NP)]
            s_all = persist.tile([128, HPC * NIC], F32, tag="s_all")
            r_all = persist.tile([128, HPC * NIC], F32, tag="r_all")
            s_part = persist.tile([128, HPC * NIC * NT], F32, tag="s_part")
            r_d = rdram.tile([HPC * NIC, 128], F32)

            for t in range(NP):
                nc.sync.dma_start(woT_sb[t][:], woT[t * 128:(t + 1) * 128, :])

            # ---- projections ----
            for pname, w_dram, x_dram in (
                    ("q", wqT, qT), ("k", wkT, kT), ("v", wvT, vT)):
                wts, xts = [], []
                for kc in range(NKC):
                    wt = wstream.tile([128, ES], F32R, tag="w", name="wt")
                    nc.sync.dma_start(wt[:], w_dram[kc * 128:(kc + 1) * 128, :])
                    wts.append(wt)
                    xt = xstream.tile([128, L], F32R, tag="x", name="xt")
                    nc.sync.dma_start(xt[:], x_dram[kc * 128:(kc + 1) * 128, :])
                    xts.append(xt)
                if pname in ("q", "k"):
                    dest = qsT if pname == "q" else ksT
                    bias = bqs_sb if pname == "q" else bk_sb
                    scale = SCALE if pname == "q" else 1.0
                    for mc in range(NP):
                        for nt in range(NT):
                            ps = ps_pool.tile([128, 512], F32, tag="ps", name="ps")
                            for kc in range(NKC):
                                nc.tensor.matmul(
                                    ps[:],
                                    wts[kc][:, mc * 128:(mc + 1) * 128],
                                    xts[kc][:, nt * 512:(nt + 1) * 512],
                                    start=(kc == 0), stop=(kc == NKC - 1))
                            nc.scalar.activation(
                                dest[mc][:, nt * 512:(nt + 1) * 512], ps[:],
                                IDENT, bias=bias[:, mc:mc + 1], scale=scale)
                else:
                    for mc in range(NIC):
                        ps = ps_pool.tile([128, 512], F32, tag="ps", name="ps")
                        for kc in range(NKC):
                            nc.tensor.matmul(
                                ps[:],
                                xts[kc][:, mc * 128:(mc + 1) * 128],
                                wts[kc][:],
                                start=(kc == 0), stop=False)
                        nc.tensor.matmul(
                            ps[:], ones1[:], bv_row[:],
                            start=False, stop=True)
                        nc.scalar.activation(vs[mc][:], ps[:], IDENT)

            # ---- attention ----
            for t in range(NP):
                for hp in range(2):
                    h = 2 * t + hp
                    prow = slice(64 * hp, 64 * hp + 64)
                    # normal layout: logits -> exp(+sum) -> normalize -> attn out
                    for ic in range(NIC):
                        u = h * NIC + ic
                        a_st = attn_st.tile([128, L], F32, tag="attn", name="a_st")
                        eps = []
                        for nt in range(NT):
                            ps = ps_pool.tile([128, 512], F32, tag="ps", name="ps")
                            nc.tensor.matmul(
                                ps[:],
                                qsT[t][prow, ic * 128:(ic + 1) * 128],
                                ksT[t][prow, nt * 512:(nt + 1) * 512],
                                start=True, stop=False)
                            nc.tensor.matmul(
                                ps[:], ones1[:],
                                madd_row[:, nt * 512:(nt + 1) * 512],
                                start=False, stop=True)
                            ep = expP_pool.tile([128, 512], F32, tag="expP",
                                                name="ep")
                            nc.scalar.activation(
                                ep[:], ps[:], EXP,
                                accum_out=s_part[:, 2 * u + nt:2 * u + nt + 1])
                            eps.append(ep)
                        nc.vector.tensor_add(
                            s_all[:, u:u + 1], s_part[:, 2 * u:2 * u + 1],
                            s_part[:, 2 * u + 1:2 * u + 2])
                        nc.vector.reciprocal(
                            r_all[:, u:u + 1], s_all[:, u:u + 1])
                        for nt in range(NT):
                            nc.vector.tensor_scalar_mul(
                                a_st[:, nt * 512:(nt + 1) * 512], eps[nt][:],
                                r_all[:, u:u + 1])
                        nc.sync.dma_start(
                            attn_o[h, ic * 128:(ic + 1) * 128, :], a_st[:])
                    # transpose this head's 1/s row block to [8, 128] layout
                    pt = ps_pool.tile([8, 128], F32, tag="ps", name="pt")
                    nc.tensor.transpose(
                        pt[:], r_all[:, h * NIC:(h + 1) * NIC], ident[:])
                    rT_sb = rbc_pool.tile([8, 128], F32, tag="rT", name="rT_sb")
                    nc.vector.tensor_copy(rT_sb[:], pt[:])
                    nc.sync.dma_start(
                        r_d[h * NIC:(h + 1) * NIC, :], rT_sb[:])

                # transposed path for the pair: logitsT -> expT -> attn@V
                for nt in range(NT):
                    outU = [acc_pool.tile([64, 512], F32, tag="acc",
                                          name=f"outU{hp}") for hp in range(2)]
                    for jc in range(NIC):
                        psA = ps_pool.tile([128, 512], F32, tag="ps", name="psA")
                        psB = ps_pool.tile([128, 512], F32, tag="ps", name="psB")
                        nc.tensor.matmul(
                            psA[:],
                            ksT[t][0:64, jc * 128:(jc + 1) * 128],
                            qsT[t][0:64, nt * 512:(nt + 1) * 512],
                            start=True, stop=True)
                        nc.tensor.matmul(
                            psB[:],
                            ksT[t][64:128, jc * 128:(jc + 1) * 128],
                            qsT[t][64:128, nt * 512:(nt + 1) * 512],
                            start=True, stop=True)
                        eA = expT_pool.tile([128, 512], F32R, tag="expT",
                                            name="eA")
                        eB = expT_pool.tile([128, 512], F32R, tag="expT",
                                            name="eB")
                        nc.scalar.activation(
                            eA[:], psA[:], EXP, bias=madd_col[:, jc:jc + 1])
                        nc.scalar.activation(
                            eB[:], psB[:], EXP, bias=madd_col[:, jc:jc + 1])
                        nc.tensor.matmul(
                            outU[0][:],
                            vs[jc][:, 128 * t:128 * t + 64],
                            eA[:], start=(jc == 0), stop=(jc == NIC - 1))
                        nc.tensor.matmul(
                            outU[1][:],
                            vs[jc][:, 128 * t + 64:128 * t + 128],
                            eB[:], start=(jc == 0), stop=(jc == NIC - 1))
                    for hp in range(2):
                        h = 2 * t + hp
                        rbc = rbc_pool.tile([64, 512], F32, tag="rbc",
                                            name="rbc")
                        src = r_d[h * NIC + nt * 4:h * NIC + nt * 4 + 4, :]
                        nc.sync.dma_start(rbc[:], _bcast_ap(src, 64))
                        of = fin_pool.tile([64, 512], F32, tag="of", name="of")
                        nc.vector.tensor_mul(of[:], outU[hp][:], rbc[:])
                        nc.gpsimd.dma_start(
                            outTn[t][64 * hp:64 * hp + 64,
                                     nt * 512:(nt + 1) * 512], of[:])

            # ---- final projection (partial over the local embed slice) ----
            for ic in range(NIC):
                for ft in range(NT):
                    ps = acc_pool.tile([128, 512], F32, tag="acc", name="psf")
                    for t in range(NP):
                        nc.tensor.matmul(
                            ps[:],
                            outTn[t][:, ic * 128:(ic + 1) * 128],
                            woT_sb[t][:, ft * 512:(ft + 1) * 512],
                            start=(t == 0), stop=False)
                    nc.tensor.matmul(
                        ps[:], ones1[:],
                        bo_row[:, ft * 512:(ft + 1) * 512],
                        start=False, stop=True)
                    f_st = fin_pool.tile([128, 512], F32, tag="fin", name="f_st")
                    nc.vector.tensor_copy(f_st[:], ps[:])
                    nc.sync.dma_start(
                        final_o[ic * 128:(ic + 1) * 128,
                                ft * 512:(ft + 1) * 512], f_st[:])

    nc.compile()
    return nc


_NC_CACHE = {}


def _get_nc():
    if "nc" not in _NC_CACHE:
        _NC_CACHE["nc"] = _build()
    return _NC_CACHE["nc"]


def make_in_maps(q, k, v, attention_mask, Wq, bq, Wk, bk, Wv, bv, Wo, bo):
    q, k, v = (np.asarray(x, np.float32) for x in (q, k, v))
    Wq, Wk, Wv, Wo = (np.asarray(x, np.float32) for x in (Wq, Wk, Wv, Wo))
    bq, bk, bv, bo = (np.asarray(x, np.float32) for x in (bq, bk, bv, bo))
    madd = np.where(np.asarray(attention_mask) == 0, NEG,
                    np.float32(0.0)).astype(np.float32)
    zeros_E = np.zeros(E, np.float32)
    ones128 = np.ones(128, np.float32)
    in_maps = []
    for c in range(8):
        b, half = divmod(c, 2)
        es = slice(half * ES, half * ES + ES)
        in_maps.append({
            "qT": np.ascontiguousarray(q[b].T),
            "kT": np.ascontiguousarray(k[b].T),
            "vT": np.ascontiguousarray(v[b].T),
            "wqT": np.ascontiguousarray(Wq[es, :].T),
            "wkT": np.ascontiguousarray(Wk[es, :].T),
            "wvT": np.ascontiguousarray(Wv[es, :].T),
            "woT": np.ascontiguousarray(Wo[:, es].T),
            "bq": np.ascontiguousarray(bq[es]),
            "bk": np.ascontiguousarray(bk[es]),
            "bv": np.ascontiguousarray(bv[es]),
            "bo": bo if half == 0 else zeros_E,
            "maddp": madd[b],
            "maddp_r": madd[b],
            "ones_d": ones128,
        })
    return in_maps


def assemble(results):
    attn = np.empty((B, H, L, L), np.float32)
    out = np.empty((B, L, E), np.float32)
    for c, r in enumerate(results):
        b, half = divmod(c, 2)
        attn[b, half * HPC:(half + 1) * HPC] = r["attn_o"]
        if half == 0:
            out[b] = r["final_o"]
        else:
            out[b] += r["final_o"]
    return out, attn


def kernel(q, k, v, attention_mask, Wq, bq, Wk, bk, Wv, bv, Wo, bo):
    nc = _get_nc()
    in_maps = make_in_maps(q, k, v, attention_mask,
                           Wq, bq, Wk, bk, Wv, bv, Wo, bo)
    res = run_bass_kernel_spmd(nc, in_maps, list(range(8)))
    return assemble(res.results)
